# revision 5
# baseline (speedup 1.0000x reference)
"""SS2D CrossBlock kernel v3 for 8 NeuronCores (Trainium2).

v3 over v2: packed weight loads (2 DMAs), conv as ACT-scale + DVE-add,
B/C replication via PE matmul, phase-3 prep in direction pairs with
batched Exp/Ln tables, 1024-col bf16 matmuls, prep(0,2) overlapped
under the k=1 scans, leaner tail (SBUF-SBUF reshape DMAs, bf16 output).
"""
import numpy as np
from contextlib import ExitStack
BF_NP = np.float16

import concourse.bass as bass
import concourse.bacc as bacc_mod
import concourse.tile as tile
from concourse import mybir
from concourse.bass_utils import run_bass_kernel_spmd

F32 = mybir.dt.float32
BF = mybir.dt.float16
AF = mybir.ActivationFunctionType
OP = mybir.AluOpType

B, HH, WW, DM = 4, 32, 32, 96
DI, NS, RD, K, L = 192, 16, 6, 4, 1024
DH = 96
NT = DH // 8
EPS = 1e-5
PADL = 34 * 34 + 4
W9 = 32 * 34

# packed bf16 weight layout: name -> (col offset, ncols, npart)
BCOLS = [
    ("xpw", 256, 96), ("wdt", 768, 96), ("bcm", 1536, 96),
    ("red", 1152, 128), ("w_g", 96, 96), ("w_b", 96, 96),
    ("ones2", 1, 96), ("sel2", 192, 2), ("bcsel", 256, 32),
]
FCOLS = [
    ("convw", 18, 96), ("convb", 2, 96), ("dtb", 4, 96),
    ("app", 48, 128), ("dsum", 1, 96),
]
BOFF = {}
off = 0
for n, c, p in BCOLS:
    BOFF[n] = (off, c, p)
    off += c
BTOT = off
FOFF = {}
off = 0
for n, c, p in FCOLS:
    FOFF[n] = (off, c, p)
    off += c
FTOT = off

_NC = None


def build():
    nc = bacc_mod.Bacc(trn_type="TRN2", target_bir_lowering=False,
                       debug=False, num_devices=8)

    xTd = nc.dram_tensor("xT", [DM, L], BF, kind="ExternalInput")
    wind = nc.dram_tensor("winp", [DM, DI + DH], BF, kind="ExternalInput")
    wpackd = nc.dram_tensor("wpackb", [128, BTOT], BF, kind="ExternalInput")
    fpackd = nc.dram_tensor("fpackf", [128, FTOT], F32, kind="ExternalInput")
    out_part = nc.dram_tensor("out_part", [DM, L], BF, kind="ExternalOutput")
    warm_in = nc.dram_tensor("warm_in", [1, 8], F32)
    warm_out = nc.dram_tensor("warm_out", [1, 8], F32)
    stats_in = nc.dram_tensor("stats_in", [2, L], F32)
    stats_out = nc.dram_tensor("stats_out", [2, L], F32)
    minv_dram = nc.dram_tensor("minv_dram", [2, L], F32)
    groups = [[0, 1], [2, 3], [4, 5], [6, 7]]

    with tile.TileContext(nc) as tc, ExitStack() as ctx:
        wpool = ctx.enter_context(tc.tile_pool(name="w", bufs=1))
        spool = ctx.enter_context(tc.tile_pool(name="s", bufs=1))
        kpool = ctx.enter_context(tc.tile_pool(name="kk", bufs=2))
        cpool = ctx.enter_context(tc.tile_pool(name="cc", bufs=3))
        tpool = ctx.enter_context(tc.tile_pool(name="t", bufs=3))
        bpool = ctx.enter_context(tc.tile_pool(name="bb", bufs=1, space="PSUM"))
        ypool = ctx.enter_context(tc.tile_pool(name="yy", bufs=1, space="PSUM"))

        xT_t = wpool.tile([DM, L], BF, name="xT_t")
        nc.sync.dma_start(xT_t[:], xTd[:])
        win_t = wpool.tile([DM, DI + DH], BF, name="win_t")
        nc.sync.dma_start(win_t[:], wind[:])
        warm_sb = wpool.tile([1, 8], F32, name="warm_sb")
        nc.vector.memset(warm_sb[:], 0.0)
        nc.sync.dma_start(warm_in[:], warm_sb[:])
        nc.gpsimd.collective_compute(
            "AllReduce", OP.add, replica_groups=groups,
            ins=[warm_in[:]], outs=[warm_out[:]])
        wb = wpool.tile([128, BTOT], BF, name="wb")
        nc.scalar.dma_start(wb[:], wpackd[:])
        wf = wpool.tile([128, FTOT], F32, name="wf")
        nc.sync.dma_start(wf[:], fpackd[:])

        def wbv(name):
            o, c, p = BOFF[name]
            return wb[0:p, o:o + c]

        def wfv(name):
            o, c, p = FOFF[name]
            return wf[0:p, o:o + c]

        xT_sb = xT_t[:]
        w_xi_sb = win_t[:, 0:DI]
        w_z_sb = win_t[:, DI:DI + DH]
        xpw_sb = wbv("xpw")
        wdt_sb = wbv("wdt")
        bcm_sb = wbv("bcm")
        red_sb = wbv("red")
        w_g_sb = wbv("w_g")
        w_b_sb = wbv("w_b")
        ones_sb = wbv("ones2")
        sel2_sb = wbv("sel2")
        bcsel_sb = wbv("bcsel")
        convw_sb = wfv("convw")
        convb_sb = wfv("convb")
        dtb_sb = wfv("dtb")
        app_sb = wfv("app")
        dsum_sb = wfv("dsum")

        def pbcd(name):
            return bpool.tile([128, L], F32, tag="bcd", name=name)

        def pbcu(name):
            return bpool.tile([128, L], F32, tag="bcu", name=name)

        def pscr(name):
            return ypool.tile([128, L], F32, tag="yhm", name=name)

        def mm(out, lhsT, rhs, start=True, stop=True):
            """1024-col matmul as two 512-col bank-sized matmuls."""
            for hh in range(2):
                nc.tensor.matmul(out[:, hh * 512:(hh + 1) * 512], lhsT,
                                 rhs[:, hh * 512:(hh + 1) * 512],
                                 start=start, stop=stop)

        # ---- phase 1: in_proj (1024-col bf16 matmuls) ----
        sg = spool.tile([DH, L], BF, name="sg")
        xpad = [spool.tile([DH, PADL], BF, name=f"xpad{i}") for i in range(2)]
        for cblk in range(2):
            nc.vector.memset(xpad[cblk][:], 0.0)
        for cblk in range(2):
            ps = pbcd(f"xi{cblk}") if cblk == 0 else pbcu(f"xi{cblk}")
            psv = ps[0:DH, :]
            mm(psv, w_xi_sb[:, cblk * DH:(cblk + 1) * DH], xT_sb)
            dst = xpad[cblk][:, 35:35 + W9]
            dstv = dst.rearrange("p (r c) -> p r c", r=32, c=34)[:, :, 0:32]
            src = psv.rearrange("p (r c) -> p r c", r=32, c=32)
            nc.scalar.activation(dstv, src, AF.Copy)
        psz = pbcd("z")
        mm(psz[0:DH, :], w_z_sb, xT_sb)
        nc.scalar.activation(sg[:], psz[0:DH, :], AF.Silu)

        # ---- phase 2: conv; cblk0 on DVE STT chain, cblk1 via ACT+adds ----
        xc = [spool.tile([DH, L], BF, name=f"xc{i}") for i in range(2)]
        xc_wm = [spool.tile([DH, L], BF, name=f"xcw{i}") for i in range(2)]
        accs = [None, None]
        # cblk1 tap scale-copies on ACT (independent of cblk0 DVE work)
        taps1 = []
        for tap in range(9):
            dy, dx = tap // 3, tap % 3
            off_ = dy * 34 + dx
            s_t = cpool.tile([DH, W9], BF, tag=f"tp{tap % 3}", name=f"s1{tap}")
            bias = convb_sb[:, 1:2] if tap == 0 else 0.0
            fn = AF.Identity if tap == 0 else AF.Copy
            nc.scalar.activation(s_t[:], xpad[1][:, off_:off_ + W9],
                                 fn, bias=bias,
                                 scale=convw_sb[:, 9 + tap:9 + tap + 1])
            taps1.append(s_t)
        # cblk0: STT chain on DVE
        acc = kpool.tile([DH, W9], BF, tag="cacc", name="m0_0")
        nc.vector.tensor_scalar(acc[:], xpad[0][:, 35:35 + W9],
                                convw_sb[:, 4:5], convb_sb[:, 0:1],
                                OP.mult, OP.add)
        for tap in range(9):
            if tap == 4:
                continue
            dy, dx = tap // 3, tap % 3
            off_ = dy * 34 + dx
            acc2 = kpool.tile([DH, W9], BF, tag="cacc", name=f"m0_{tap}")
            nc.vector.scalar_tensor_tensor(
                acc2[:], xpad[0][:, off_:off_ + W9],
                convw_sb[:, tap:tap + 1], acc[:], OP.mult, OP.add)
            acc = acc2
        accs[0] = acc
        # cblk1: DVE adds over ACT-scaled taps
        acc = kpool.tile([DH, W9], BF, tag="cacc", name="m1_0")
        nc.vector.tensor_tensor(acc[:], taps1[0][:], taps1[1][:], OP.add)
        for i in range(2, 9):
            acc2 = kpool.tile([DH, W9], BF, tag="cacc", name=f"m1_{i}")
            nc.vector.tensor_tensor(acc2[:], acc[:], taps1[i][:], OP.add)
            acc = acc2
        accs[1] = acc
        for cblk in range(2):
            acc = accs[cblk]
            accv = acc[:].rearrange("p (r c) -> p r c", r=32, c=34)[:, :, 0:32]
            nc.scalar.activation(
                xc[cblk][:].rearrange("p (r c) -> p r c", r=32, c=32),
                accv, AF.Silu)
            accw = acc[:].rearrange("p (r c) -> p c r", r=32, c=34)[:, 0:32, :]
            nc.scalar.activation(
                xc_wm[cblk][:].rearrange("p (w h) -> p w h", w=32, h=32),
                accw, AF.Silu)

        # ---- phase 3 helpers ----
        du = [None] * K
        bb2s = [None] * K
        cb2s = [None] * K
        esp = [None] * K

        def prep_a(k):
            src01 = xc if k in (0, 2) else xc_wm
            zk = pscr(f"zk{k}")
            zkv = zk[0:2 * NS, :]
            for cblk in range(2):
                w0 = (k * 2 + cblk) * 32
                mm(zkv, xpw_sb[:, w0:w0 + 32], src01[cblk][:],
                   start=(cblk == 0), stop=(cblk == 1))
            bck = kpool.tile([2 * NS, L], BF, tag="bck", name=f"bck{k}")
            nc.scalar.activation(bck[:], zkv, AF.Copy)
            return None, bck

        def prep_b(k, dts):
            src01 = xc if k in (0, 2) else xc_wm
            dtd = pscr(f"dtd{k}")
            for cblk in range(2):
                w0 = (k * 2 + cblk) * DH
                mm(dtd[0:DH, :], wdt_sb[:, w0:w0 + DH], src01[cblk][:],
                   start=(cblk == 0), stop=(cblk == 1))
            esp[k] = spool.tile([DH, L], F32, name=f"esp{k}")
            nc.scalar.activation(esp[k][:], dtd[0:DH, :], AF.Exp,
                                 bias=dtb_sb[:, k:k + 1], scale=1.0)

        def prep_c(k, bck):
            bb2 = spool.tile([128, L], BF, name=f"bb2_{k}")
            cb2 = spool.tile([128, L], BF, name=f"cb2_{k}")
            bps = pscr(f"bbp{k}")
            mm(bps[:], bcsel_sb[:, 0:128], bck[:])
            nc.scalar.activation(bb2[:], bps[:], AF.Copy)
            cps = pscr(f"cbp{k}")
            mm(cps[:], bcsel_sb[:, 128:256], bck[:])
            nc.scalar.activation(cb2[:], cps[:], AF.Copy)
            bb2s[k] = bb2
            cb2s[k] = cb2

        def prep_mm(k):
            dts, bck = prep_a(k)
            prep_b(k, dts)
            prep_c(k, bck)

        def fin_ln(k):
            duk = spool.tile([DH, 2 * L], BF, name=f"du{k}")
            nc.scalar.activation(duk[:, 0:L], esp[k][:], AF.Ln, bias=1.0,
                                 scale=1.0)
            du[k] = duk

        def fin_u(k):
            src01 = xc if k in (0, 2) else xc_wm
            nc.vector.tensor_tensor(du[k][:, L:2 * L], du[k][:, 0:L],
                                    src01[0][:], OP.mult)

        def prep_fin(k):
            fin_ln(k)
            fin_u(k)

        y_box = {}
        y_box['wm'] = ypool.tile([DH, L], F32, tag="ywm", name="y_wm")
        ywm_hm = spool.tile([DH, L], BF, name="ywm_hm")

        def scan_dir(k, chunks=()):
            wm = k in (1, 3)
            flip = k >= 2
            y_ps = y_box['wm'] if wm else y_box['hm']
            for t in range(NT):
                if t % 2 == 0 and t // 2 < len(chunks):
                    chunks[t // 2]()
                bcd = pbcd(f"bcd{k}_{t}")
                mm(bcd[:], bcm_sb[:, t * 128:(t + 1) * 128], du[k][:, 0:L])
                a_t = tpool.tile([128, L], BF, tag="a", name=f"a{k}_{t}")
                scl = app_sb[:, k * NT + t:k * NT + t + 1]
                nc.scalar.activation(a_t[:], bcd[:], AF.Exp,
                                     bias=0.0, scale=scl)
                bcu = pbcu(f"bcu{k}_{t}")
                mm(bcu[:], bcm_sb[:, t * 128:(t + 1) * 128],
                   du[k][:, L:2 * L])
                u_t = tpool.tile([128, L], BF, tag="u", name=f"u{k}_{t}")
                nc.scalar.activation(u_t[:], bcu[:], AF.Copy)
                b_t = tpool.tile([128, L], BF, tag="b", name=f"b{k}_{t}")
                nc.vector.tensor_tensor(b_t[:], u_t[:], bb2s[k][:], OP.mult)
                h_t = tpool.tile([128, L], BF, tag="h", name=f"h{k}_{t}")
                if flip:
                    nc.vector.tensor_tensor_scan(
                        h_t[:, ::-1], a_t[:, ::-1], b_t[:, ::-1], 0.0,
                        OP.mult, OP.add)
                else:
                    nc.vector.tensor_tensor_scan(
                        h_t[:], a_t[:], b_t[:], 0.0, OP.mult, OP.add)
                hc_t = tpool.tile([128, L], BF, tag="hc", name=f"hc{k}_{t}")
                nc.vector.tensor_tensor(hc_t[:], h_t[:], cb2s[k][:], OP.mult)
                for hh in range(2):
                    nc.tensor.matmul(
                        y_ps[:, hh * 512:(hh + 1) * 512],
                        red_sb[:, t * DH:(t + 1) * DH],
                        hc_t[:, hh * 512:(hh + 1) * 512],
                        start=(k in (0, 1) and t == 0),
                        stop=(k in (2, 3) and t == NT - 1))

        # head: only prep(1); others sprinkle into the scan loops
        prep_mm(1)
        prep_fin(1)
        st3 = {}
        ch1 = [
            lambda: st3.update(x=prep_a(3)),
            lambda: prep_b(3, st3['x'][0]),
            lambda: prep_c(3, st3['x'][1]),
            lambda: st3.update(y=prep_a(0)),
            lambda: (prep_b(0, st3['y'][0]), prep_c(0, st3['y'][1])),
            lambda: (fin_ln(3), fin_ln(0)),
        ]
        scan_dir(1, ch1)
        st2_ = {}
        ch3 = [
            lambda: (fin_u(3), fin_u(0)),
            lambda: st2_.update(x=prep_a(2)),
            lambda: (prep_b(2, st2_['x'][0]), prep_c(2, st2_['x'][1])),
            lambda: fin_ln(2),
        ]
        scan_dir(3, ch3)
        y_box['hm'] = ypool.tile([DH, L], F32, tag="yhm", name="y_hm")
        # y_wm done: permute to h-major while k=0,2 still scan
        nc.scalar.activation(
            ywm_hm[:].rearrange("p (h w) -> p h w", h=32, w=32),
            y_box['wm'][:].rearrange("p (w h) -> p h w", w=32, h=32),
            AF.Copy)
        P2 = ypool.tile([DH, L], F32, tag="ywm", name="P2")
        mm(P2[:], w_g_sb, sg[:])
        P2sb = spool.tile([DH, L], BF, name="P2sb")
        nc.scalar.activation(P2sb[:], P2[:], AF.Copy)
        scan_dir(0, [lambda: fin_u(2)])
        scan_dir(2)

        # ---- phase 5: combine, LN stats, collective, out_proj ----
        yq = spool.tile([DH, L], BF, name="yq")
        nc.vector.scalar_tensor_tensor(yq[:], xc[0][:], dsum_sb,
                                       y_box['hm'][:], OP.mult, OP.add)
        y_full = spool.tile([DH, L], BF, name="y_full")
        nc.vector.tensor_tensor(y_full[:], yq[:], ywm_hm[:], OP.add)
        y2 = spool.tile([DH, L], BF, name="y2")
        nc.scalar.activation(y2[:], y_full[:], AF.Square)
        sgy = spool.tile([DH, L], BF, name="sgy")
        nc.vector.tensor_tensor(sgy[:], y_full[:], sg[:], OP.mult)

        st_y = spool.tile([1, L], F32, name="st_y")
        st_y2 = spool.tile([1, L], F32, name="st_y2")
        for row, (src_t, dst_t) in enumerate(((y_full, st_y), (y2, st_y2))):
            ssp = pbcd(f"st{row}") if row == 0 else pbcu(f"st{row}")
            sspv = ssp[0:1, :]
            mm(sspv, ones_sb, src_t[:])
            nc.scalar.activation(dst_t[:], sspv, AF.Copy)
        nc.sync.dma_start(stats_in[0:1, :], st_y[:])
        nc.sync.dma_start(stats_in[1:2, :], st_y2[:])
        nc.gpsimd.collective_compute(
            "AllReduce", OP.add, replica_groups=groups,
            ins=[stats_in[:]], outs=[stats_out[:]])

        # P1/P3 matmuls + SBUF copies (run during the collective)
        P1 = ypool.tile([DH, L], F32, tag="yhm", name="P1")
        mm(P1[:], w_g_sb, sgy[:])
        P1sb = spool.tile([DH, L], BF, name="P1sb")
        nc.scalar.activation(P1sb[:], P1[:], AF.Copy)
        p3p = pbcd("P3p")
        mm(p3p[0:DH, :], w_b_sb, sg[:])
        p3sb = spool.tile([DH, L], BF, name="p3sb")
        nc.scalar.activation(p3sb[:], p3p[0:DH, :], AF.Copy)

        # stats back: reshape [2,1024] -> [128,16] straight from DRAM
        st128 = spool.tile([128, 16], F32, name="st128")
        nc.sync.dma_start(
            st128[:].rearrange("p (s f) -> p s f", s=2, f=8),
            stats_out[:].rearrange("s (p f) -> p s f", p=128, f=8))
        mu8 = spool.tile([128, 8], F32, name="mu8")
        nc.scalar.activation(mu8[:], st128[:, 0:8], AF.Copy, scale=1.0 / DI)
        msq = spool.tile([128, 8], F32, name="msq")
        nc.scalar.activation(msq[:], st128[:, 0:8], AF.Square, scale=1.0 / DI)
        var8 = spool.tile([128, 8], F32, name="var8")
        nc.vector.scalar_tensor_tensor(var8[:], st128[:, 8:16], 1.0 / DI,
                                       msq[:], OP.mult, OP.subtract)
        eps_sb = spool.tile([128, 1], F32, name="eps_sb")
        nc.vector.memset(eps_sb[:], EPS)
        sd8 = spool.tile([128, 8], F32, name="sd8")
        nc.scalar.activation(sd8[:], var8[:], AF.Sqrt, bias=eps_sb[:],
                             scale=1.0)
        ii8 = spool.tile([128, 16], F32, name="ii8")
        nc.vector.reciprocal(ii8[:, 0:8], sd8[:])
        nc.vector.tensor_tensor(ii8[:, 8:16], mu8[:], ii8[:, 0:8], OP.mult)
        nc.sync.dma_start(
            minv_dram[:].rearrange("s (p f) -> p s f", p=128, f=8),
            ii8[:].rearrange("p (s f) -> p s f", s=2, f=8))
        minv_f = spool.tile([2, L], F32, name="minv_f")
        nc.sync.dma_start(minv_f[:], minv_dram[:])
        minv = spool.tile([2, L], BF, name="minv")
        nc.vector.tensor_copy(minv[:], minv_f[:])
        ib_sb = spool.tile([DH, L], BF, name="ib_sb")
        imub_sb = spool.tile([DH, L], BF, name="imub_sb")
        bb1 = pbcd("ib")
        mm(bb1[0:DH, :], sel2_sb[:, 0:DH], minv[:])
        nc.scalar.activation(ib_sb[:], bb1[0:DH, :], AF.Copy)
        bb2p = pbcu("imub")
        mm(bb2p[0:DH, :], sel2_sb[:, DH:2 * DH], minv[:])
        nc.scalar.activation(imub_sb[:], bb2p[0:DH, :], AF.Copy)
        q1 = spool.tile([DH, L], BF, name="q1")
        nc.vector.tensor_tensor(q1[:], P1sb[:], ib_sb[:], OP.mult)
        q2 = spool.tile([DH, L], BF, name="q2")
        nc.vector.tensor_tensor(q2[:], P2sb[:], imub_sb[:], OP.mult)
        q3 = spool.tile([DH, L], BF, name="q3")
        nc.vector.tensor_tensor(q3[:], q1[:], q2[:], OP.subtract)
        outf = spool.tile([DM, L], BF, name="outf")
        for hh in range(2):
            sl = slice(hh * 512, (hh + 1) * 512)
            nc.vector.tensor_tensor(outf[:, sl], q3[:, sl], p3sb[:, sl],
                                    OP.add)
            nc.sync.dma_start(out_part[:, sl], outf[:, sl])

    nc.finalize()
    return nc


def _prep_inputs(inputs):
    x = np.asarray(inputs["x"], np.float32)
    in_proj_w = np.asarray(inputs["in_proj_w"], np.float32)
    conv_w = np.asarray(inputs["conv_w"], np.float32)
    conv_b = np.asarray(inputs["conv_b"], np.float32)
    xpw = np.asarray(inputs["x_proj_weight"], np.float32)
    dtw = np.asarray(inputs["dt_projs_weight"], np.float32)
    dtb = np.asarray(inputs["dt_projs_bias"], np.float32)
    A_logs = np.asarray(inputs["A_logs"], np.float32)
    Ds = np.asarray(inputs["Ds"], np.float32)
    gam = np.asarray(inputs["ln_gamma"], np.float32)
    bet = np.asarray(inputs["ln_beta"], np.float32)
    wout = np.asarray(inputs["out_proj_w"], np.float32)

    xTf = x.reshape(B, L, DM).transpose(0, 2, 1).copy()
    w_in_T = in_proj_w.T.copy()
    convw9 = conv_w.reshape(DI, 9)
    A = -np.exp(A_logs).reshape(K, DI, NS)
    Dsum_full = Ds.reshape(K, DI).sum(0)

    bcm = np.zeros((DH, NT * 128), np.float32)
    for t in range(NT):
        for j in range(128):
            bcm[8 * t + j // 16, t * 128 + j] = 1.0
    red = np.zeros((128, NT * DH), np.float32)
    for t in range(NT):
        for j in range(128):
            red[j, t * DH + 8 * t + j // 16] = 1.0
    sel2 = np.zeros((2, 2 * DH), np.float32)
    sel2[0, 0:DH] = 1.0
    sel2[1, DH:2 * DH] = 1.0
    bcsel = np.zeros((32, 256), np.float32)
    for j in range(128):
        bcsel[j % 16, j] = 1.0
        bcsel[16 + j % 16, 128 + j] = 1.0

    in_maps = []
    for c in range(8):
        b, dh = c // 2, c % 2
        ds = slice(dh * DH, (dh + 1) * DH)
        other = slice((1 - dh) * DH, (2 - dh) * DH)
        w_xi_r = np.concatenate([w_in_T[:, ds], w_in_T[:, other]], axis=1)
        convw_r = np.concatenate([convw9[ds], convw9[other]], axis=1)
        convb_r = np.stack([conv_b[ds], conv_b[other]], axis=1)
        xpw_r = np.zeros((DH, K * 2 * 32), np.float32)
        wdt_r = np.zeros((DH, K * 2 * DH), np.float32)
        for k in range(K):
            wk = xpw[k].T
            for cblk, sl in enumerate((ds, other)):
                w0 = (k * 2 + cblk) * 32
                xpw_r[:, w0:w0 + 2 * NS] = wk[sl][:, RD:RD + 2 * NS]
                # wdt[c, d] = sum_r xpw[k][r, c] * dtw[k][d, r]
                wdt_r[:, (k * 2 + cblk) * DH:(k * 2 + cblk + 1) * DH] = (
                    xpw[k][0:RD, sl].T @ dtw[k, ds, :].T)
        dtb_r = dtb.reshape(K, DI)[:, ds].T.copy()
        app = np.zeros((128, K * NT), np.float32)
        for k in range(K):
            for t in range(NT):
                for j in range(128):
                    app[j, k * NT + t] = A[k, dh * DH + 8 * t + j // 16, j % 16]
        w_g = wout[:, ds].T * gam[ds][:, None]
        w_b = wout[:, ds].T * bet[ds][:, None]

        bpack = np.zeros((128, BTOT), np.float32)
        fpack = np.zeros((128, FTOT), np.float32)

        def put(pack, offmap, name, arr):
            o, ccols, p = offmap[name]
            arr = np.asarray(arr, np.float32)
            if arr.ndim == 1:
                arr = arr[:, None]
            assert arr.shape == (p, ccols), (name, arr.shape, (p, ccols))
            pack[0:p, o:o + ccols] = arr

        put(bpack, BOFF, "xpw", xpw_r)
        put(bpack, BOFF, "wdt", wdt_r)
        put(bpack, BOFF, "bcm", bcm)
        put(bpack, BOFF, "red", red)
        put(bpack, BOFF, "w_g", w_g)
        put(bpack, BOFF, "w_b", w_b)
        put(bpack, BOFF, "ones2", np.ones((DH, 1), np.float32))
        put(bpack, BOFF, "sel2", sel2)
        put(bpack, BOFF, "bcsel", bcsel)
        put(fpack, FOFF, "convw", convw_r)
        put(fpack, FOFF, "convb", convb_r)
        put(fpack, FOFF, "dtb", dtb_r)
        put(fpack, FOFF, "app", app)
        put(fpack, FOFF, "dsum", Dsum_full[ds])

        winp = np.concatenate(
            [w_xi_r, w_in_T[:, DI + dh * DH: DI + (dh + 1) * DH]], axis=1)
        in_maps.append({
            "xT": xTf[b].astype(BF_NP),
            "winp": winp.astype(BF_NP),
            "wpackb": bpack.astype(BF_NP),
            "fpackf": fpack,
        })
    return in_maps


def kernel(**inputs):
    global _NC
    if _NC is None:
        _NC = build()
    in_maps = _prep_inputs(inputs)
    res = run_bass_kernel_spmd(_NC, in_maps, list(range(8)))
    out = np.zeros((B, L, DM), np.float32)
    for b in range(B):
        part = (res.results[2 * b]["out_part"].astype(np.float32)
                + res.results[2 * b + 1]["out_part"].astype(np.float32))
        out[b] = part.T
    return out.reshape(B, HH, WW, DM)
